# Initial kernel scaffold
#
"""CrossScaleVFE kernel for 8x Trainium2 NeuronCores.

Math (per branch, belief/model):
  total = sum_{n,m} W[n,m] * KL( N(mu_c[n], Sg_c[n]) || N(T mu_p[m], T Sg_p[m] T^T) ),
  T = Om_c[n] @ Om_p[m]^-1.

Reduction used here (G = Om_c^-1, c = G mu_c, B = G Sg_c G^T, S = B + c c^T):
  KL[n,m] = 0.5*( <A_m, S_n> - 2 e_m . c_n + d_m - K + f_m + g_n - h_n )
  A_m = Om_p^T Sg_p^-1 Om_p, e_m = Om_p^T Sg_p^-1 mu_p, d_m = mu_p^T Sg_p^-1 mu_p,
  f_m = logdet Sg_p - 2 log|det Om_p|, g_n = 2 log|det Om_c|, h_n = logdet Sg_c.

Sharding: child axis N split over 8 cores (256 children/core); parent-derived
constants (64 parents) replicated. Device computes, per core:
  - batched 16x16 Gauss-Jordan of [Om | Sg | mu] -> G, Z=G Sg, c=G mu + det(Om)
  - batched LU pivots of Sg -> det(Sg)
  - S = Z G^T + c c^T; PE matmul P = W^T [S | c]; out_m = reduce(P * [A | -2e])
  - out_lane = rowsum(W) * (g - h)
Host does the tiny per-parent precompute and final scalar assembly.
"""
import numpy as np

N, M, K, NC = 2048, 64, 16, 8
NS = N // NC          # children per core
P = 128               # partitions
A2 = NS // P          # child chunks per branch (2)
CH = 2 * A2           # total chunks (belief a0,a1, model a0,a1)
AUG = 2 * K + 1       # 33: [Om | Sg | mu]
SW = K * K + K        # 272: [S_flat | c]

_PROG = None


def _build_program():
    import concourse.bass as bass
    import concourse.bacc as bacc
    import concourse.tile as tile
    import concourse.mybir as mybir

    f32 = mybir.dt.float32
    Alu = mybir.AluOpType
    Act = mybir.ActivationFunctionType

    nc = bacc.Bacc("TRN2", target_bir_lowering=False, debug=False, num_devices=NC)

    omg = nc.dram_tensor("omg", (NS, K, K), f32, kind="ExternalInput").ap()
    omm = nc.dram_tensor("omm", (NS, K, K), f32, kind="ExternalInput").ap()
    sgp = nc.dram_tensor("sgp", (NS, K, K), f32, kind="ExternalInput").ap()
    sgr = nc.dram_tensor("sgr", (NS, K, K), f32, kind="ExternalInput").ap()
    mup = nc.dram_tensor("mup", (NS, K), f32, kind="ExternalInput").ap()
    mur = nc.dram_tensor("mur", (NS, K), f32, kind="ExternalInput").ap()
    win = nc.dram_tensor("w", (NS, M), f32, kind="ExternalInput").ap()
    cst = nc.dram_tensor("cst", (2, M, SW), f32, kind="ExternalInput").ap()
    out_m = nc.dram_tensor("out_m", (2, M), f32, kind="ExternalOutput").ap()
    out_lane = nc.dram_tensor("out_lane", (P, CH), f32, kind="ExternalOutput").ap()

    with tile.TileContext(nc) as tc:
        with (
            tc.tile_pool(name="main", bufs=1) as pool,
            tc.tile_pool(name="psum", bufs=1, space=bass.MemorySpace.PSUM) as pp,
        ):
            Mt = pool.tile([P, CH, K, AUG], f32)     # [Om | Sg | mu] -> [G | Z | c]
            sT = pool.tile([P, CH, K, K], f32)       # Sg copy for LU pivots
            St = pool.tile([P, CH, SW], f32)         # [S_flat | c]
            OUTt = pool.tile([P, CH, K, AUG], f32)   # outer-product scratch
            OUT2 = pool.tile([P, CH, K - 1, K - 1], f32)
            wt = pool.tile([P, A2, M], f32)
            cstT = pool.tile([M, 2, SW], f32)
            dT = pool.tile([P, CH], f32)
            dnT = pool.tile([P, CH], f32)
            d2T = pool.tile([P, CH], f32)
            RsT = pool.tile([P, CH, AUG], f32)
            CsT = pool.tile([P, CH, K], f32)
            Cs2 = pool.tile([P, CH, K - 1], f32)
            ppO = [pool.tile([P, CH], f32, tag=f"ppO{i}") for i in range(2)]
            ppS = [pool.tile([P, CH], f32, tag=f"ppS{i}") for i in range(2)]
            gsq = pool.tile([P, CH], f32)
            rS = pool.tile([P, CH], f32)
            r3 = pool.tile([P, CH], f32)
            ghT = pool.tile([P, CH], f32)
            rowW = pool.tile([P, A2, 1], f32)
            laneP = pool.tile([P, CH], f32)
            Pm = pool.tile([M, 2, SW], f32)
            omt = pool.tile([M, 2], f32)
            ps = [pp.tile([M, SW], f32, tag=f"ps{b}") for b in range(2)]

            # ---- loads ----
            for src, br in ((omg, 0), (omm, 1)):
                nc.sync.dma_start(
                    Mt[:, 2 * br:2 * br + 2, :, 0:K],
                    src.rearrange("(a p) i j -> p a i j", p=P),
                )
            for src, br in ((sgp, 0), (sgr, 1)):
                s4 = src.rearrange("(a p) i j -> p a i j", p=P)
                nc.sync.dma_start(Mt[:, 2 * br:2 * br + 2, :, K:2 * K], s4)
                nc.sync.dma_start(sT[:, 2 * br:2 * br + 2, :, :], s4)
            for src, br in ((mup, 0), (mur, 1)):
                nc.sync.dma_start(
                    Mt[:, 2 * br:2 * br + 2, :, 2 * K],
                    src.rearrange("(a p) i -> p a i", p=P),
                )
            nc.sync.dma_start(wt[:, :, :], win.rearrange("(a p) m -> p a m", p=P))
            nc.sync.dma_start(cstT[:, :, :], cst.rearrange("b m e -> m b e"))

            nc.vector.memset(ppO[0][:, :], 1.0)
            nc.vector.memset(ppS[0][:, :], 1.0)

            # ---- Gauss-Jordan sweep on [Om | Sg | mu]: -> [G | G Sg | G mu] ----
            for i in range(K):
                piv = Mt[:, :, i, i]
                nc.vector.tensor_tensor(
                    ppO[(i + 1) % 2][:, :], ppO[i % 2][:, :], piv, Alu.mult
                )
                nc.vector.reciprocal(dT[:, :], piv)
                nc.vector.tensor_scalar_mul(dnT[:, :], dT[:, :], -1.0)
                # scaled pivot row (all 33 cols)
                nc.vector.tensor_tensor(
                    RsT[:, :, :], Mt[:, :, i, :],
                    dT[:, :].unsqueeze(2).to_broadcast([P, CH, AUG]), Alu.mult
                )
                # -d * pivot column (16 rows)
                nc.vector.tensor_tensor(
                    CsT[:, :, :], Mt[:, :, :, i],
                    dnT[:, :].unsqueeze(2).to_broadcast([P, CH, K]), Alu.mult
                )
                # rank-1 update: M -= col_i (x) Rs
                nc.vector.tensor_tensor(
                    OUTt[:, :, :, :],
                    Mt[:, :, :, i].unsqueeze(3).to_broadcast([P, CH, K, AUG]),
                    RsT[:, :, :].unsqueeze(2).to_broadcast([P, CH, K, AUG]),
                    Alu.mult,
                )
                nc.vector.tensor_tensor(
                    Mt[:, :, :, :], Mt[:, :, :, :], OUTt[:, :, :, :], Alu.subtract
                )
                nc.vector.tensor_copy(Mt[:, :, :, i], CsT[:, :, :])
                nc.vector.tensor_copy(Mt[:, :, i, :], RsT[:, :, :])
                nc.vector.tensor_copy(Mt[:, :, i, i], dT[:, :])

            # ---- LU pivot products of Sg (logdet) ----
            for i in range(K):
                piv = sT[:, :, i, i]
                nc.vector.tensor_tensor(
                    ppS[(i + 1) % 2][:, :], ppS[i % 2][:, :], piv, Alu.mult
                )
                if i == K - 1:
                    break
                r = K - 1 - i
                nc.vector.reciprocal(d2T[:, :], piv)
                nc.vector.tensor_tensor(
                    Cs2[:, :, 0:r], sT[:, :, i + 1:, i],
                    d2T[:, :].unsqueeze(2).to_broadcast([P, CH, r]), Alu.mult
                )
                nc.vector.tensor_tensor(
                    OUT2[:, :, 0:r, 0:r],
                    Cs2[:, :, 0:r].unsqueeze(3).to_broadcast([P, CH, r, r]),
                    sT[:, :, i, i + 1:].unsqueeze(2).to_broadcast([P, CH, r, r]),
                    Alu.mult,
                )
                nc.vector.tensor_tensor(
                    sT[:, :, i + 1:, i + 1:], sT[:, :, i + 1:, i + 1:],
                    OUT2[:, :, 0:r, 0:r], Alu.subtract
                )

            # ---- g - h = log(det(Om)^2 / det(Sg)) ----
            nc.vector.tensor_tensor(gsq[:, :], ppO[0][:, :], ppO[0][:, :], Alu.mult)
            nc.vector.reciprocal(rS[:, :], ppS[0][:, :])
            nc.vector.tensor_tensor(r3[:, :], gsq[:, :], rS[:, :], Alu.mult)
            nc.scalar.activation(ghT[:, :], r3[:, :], Act.Ln)

            # ---- S = Z G^T + c c^T (k-accumulation of outer products) ----
            Sv = St[:, :, 0:K * K].rearrange("p c (i j) -> p c i j", i=K)
            for k in range(K + 1):
                zo = 2 * K if k == K else K + k    # Z col k, or c
                go = 2 * K if k == K else k        # G col k, or c
                in0 = Mt[:, :, :, zo].unsqueeze(3).to_broadcast([P, CH, K, K])
                in1 = Mt[:, :, :, go].unsqueeze(2).to_broadcast([P, CH, K, K])
                if k == 0:
                    nc.vector.tensor_tensor(Sv, in0, in1, Alu.mult)
                else:
                    nc.vector.tensor_tensor(OUTt[:, :, :, 0:K], in0, in1, Alu.mult)
                    nc.vector.tensor_tensor(Sv, Sv, OUTt[:, :, :, 0:K], Alu.add)
            nc.vector.tensor_copy(St[:, :, K * K:SW], Mt[:, :, :, 2 * K])

            # ---- lane partials: rowsum(W) * (g - h) ----
            nc.vector.tensor_reduce(
                rowW[:, :, :], wt[:, :, :], mybir.AxisListType.X, Alu.add
            )
            nc.vector.tensor_tensor(
                laneP[:, :].rearrange("p (b a) -> p b a", b=2),
                ghT[:, :].rearrange("p (b a) -> p b a", b=2),
                rowW[:, :, 0].unsqueeze(1).to_broadcast([P, 2, A2]),
                Alu.mult,
            )
            nc.sync.dma_start(out_lane[:, :], laneP[:, :])

            # ---- PE: P = W^T @ [S | c], then out_m = reduce(P * [A | -2e]) ----
            for br in range(2):
                for a in range(A2):
                    nc.tensor.matmul(
                        ps[br][:, :], wt[:, a, :], St[:, 2 * br + a, :],
                        start=(a == 0), stop=(a == A2 - 1),
                    )
                nc.vector.tensor_tensor(
                    Pm[:, br, :], ps[br][:, :], cstT[:, br, :], Alu.mult
                )
                nc.vector.tensor_reduce(
                    omt[:, br:br + 1], Pm[:, br, :], mybir.AxisListType.X, Alu.add
                )
            nc.sync.dma_start(out_m.rearrange("b m -> m b"), omt[:, :])

    nc.compile()
    return nc


def _get_program():
    global _PROG
    if _PROG is None:
        _PROG = _build_program()
    return _PROG


def _parent_precompute(mu_p, sg_p, om_p):
    mu = mu_p.astype(np.float64)
    sg = sg_p.astype(np.float64)
    om = om_p.astype(np.float64)
    Si = np.linalg.inv(sg)
    A = np.einsum('mji,mjk,mkl->mil', om, Si, om)
    e = np.einsum('mji,mjk,mk->mi', om, Si, mu)
    d = np.einsum('mj,mjk,mk->m', mu, Si, mu)
    f = np.linalg.slogdet(sg)[1] - 2.0 * np.log(np.abs(np.linalg.det(om)))
    cst = np.concatenate([A.reshape(M, K * K), -2.0 * e], axis=1).astype(np.float32)
    return cst, d, f


def make_in_maps(inputs):
    """Shard inputs over cores + build replicated parent constants.
    Returns (in_maps, host_terms[2])."""
    cb, db, fb = _parent_precompute(
        inputs['mu_q_parent'], inputs['sigma_q_parent'], inputs['omega_parent'])
    cm, dm, fm = _parent_precompute(
        inputs['mu_s_parent'], inputs['sigma_s_parent'], inputs['omega_m_parent'])
    cst = np.stack([cb, cm]).astype(np.float32)                      # (2, M, SW)
    W64 = inputs['W'].astype(np.float64)
    colW = W64.sum(axis=0)
    host_terms = [float((colW * (db + fb - K)).sum()),
                  float((colW * (dm + fm - K)).sum())]

    f32c = lambda x: np.ascontiguousarray(x, dtype=np.float32)
    in_maps = []
    for c in range(NC):
        s = slice(c * NS, (c + 1) * NS)
        in_maps.append({
            'omg': f32c(inputs['omega_child'][s]),
            'omm': f32c(inputs['omega_m_child'][s]),
            'sgp': f32c(inputs['sigma_p'][s]),
            'sgr': f32c(inputs['sigma_r'][s]),
            'mup': f32c(inputs['mu_p'][s]),
            'mur': f32c(inputs['mu_r'][s]),
            'w': f32c(inputs['W'][s]),
            'cst': cst,
        })
    return in_maps, host_terms


def assemble(results, host_terms):
    vals = []
    for br in range(2):
        dev = 0.0
        for r in results:
            dev += float(r['out_m'][br].astype(np.float64).sum())
            dev += float(r['out_lane'][:, 2 * br:2 * br + 2].astype(np.float64).sum())
        vals.append(0.5 * (dev + host_terms[br]))
    belief, model = vals
    total = belief + model
    return (np.float32(total), np.float32(belief), np.float32(model))


def kernel(**inputs):
    from concourse import bass_utils
    nc = _get_program()
    in_maps, host_terms = make_in_maps(inputs)
    res = bass_utils.run_bass_kernel_spmd(nc, in_maps, core_ids=list(range(NC)))
    return assemble(res.results, host_terms)


if __name__ == "__main__":
    _build_program()
    print("program builds OK")


# revision 5
# speedup vs baseline: 1.1472x; 1.1472x over previous
"""CrossScaleVFE kernel for 8x Trainium2 NeuronCores.

Math (per branch, belief/model):
  total = sum_{n,m} W[n,m] * KL( N(mu_c[n], Sg_c[n]) || N(T mu_p[m], T Sg_p[m] T^T) ),
  T = Om_c[n] @ Om_p[m]^-1.

Reduction used here (G = Om_c^-1, c = G mu_c, B = G Sg_c G^T, S = B + c c^T):
  KL[n,m] = 0.5*( <A_m, S_n> - 2 e_m . c_n + d_m - K + f_m + g_n - h_n )
  A_m = Om_p^T Sg_p^-1 Om_p, e_m = Om_p^T Sg_p^-1 mu_p, d_m = mu_p^T Sg_p^-1 mu_p,
  f_m = logdet Sg_p - 2 log|det Om_p|, g_n = 2 log|det Om_c|, h_n = logdet Sg_c.

Sharding: child axis N split over 8 cores (256 children/core); parent-derived
constants (64 parents) replicated. Device computes, per core:
  - batched 16x16 Gauss-Jordan of [Om | Sg | mu] -> G, Z=G Sg, c=G mu + det(Om)
  - batched LU pivots of Sg -> det(Sg)
  - S = Z G^T + c c^T; PE matmul P = W^T [S | c]; out_m = reduce(P * [A | -2e])
  - out_lane = rowsum(W) * (g - h)
Host does the tiny per-parent precompute and final scalar assembly.
"""
import numpy as np

N, M, K, NC = 2048, 64, 16, 8
NS = N // NC          # children per core
P = 128               # partitions
A2 = NS // P          # child chunks per branch (2)
CH = 2 * A2           # total chunks (belief a0,a1, model a0,a1)
AUG = 2 * K + 1       # 33: [Om | Sg | mu]
SW = K * K + K        # 272: [S_flat | c]

_PROG = None


def _build_program():
    import concourse.bass as bass
    import concourse.bacc as bacc
    import concourse.tile as tile
    import concourse.mybir as mybir

    f32 = mybir.dt.float32
    Alu = mybir.AluOpType
    Act = mybir.ActivationFunctionType

    nc = bacc.Bacc("TRN2", target_bir_lowering=False, debug=False, num_devices=NC)

    omg = nc.dram_tensor("omg", (NS, K, K), f32, kind="ExternalInput").ap()
    omm = nc.dram_tensor("omm", (NS, K, K), f32, kind="ExternalInput").ap()
    sgp = nc.dram_tensor("sgp", (NS, K, K), f32, kind="ExternalInput").ap()
    sgr = nc.dram_tensor("sgr", (NS, K, K), f32, kind="ExternalInput").ap()
    mup = nc.dram_tensor("mup", (NS, K), f32, kind="ExternalInput").ap()
    mur = nc.dram_tensor("mur", (NS, K), f32, kind="ExternalInput").ap()
    win = nc.dram_tensor("w", (NS, M), f32, kind="ExternalInput").ap()
    cst = nc.dram_tensor("cst", (2, M, SW), f32, kind="ExternalInput").ap()
    out_m = nc.dram_tensor("out_m", (2, M), f32, kind="ExternalOutput").ap()
    out_lane = nc.dram_tensor("out_lane", (P, CH), f32, kind="ExternalOutput").ap()

    with tile.TileContext(nc) as tc:
        with (
            tc.tile_pool(name="main", bufs=1) as pool,
            tc.tile_pool(name="tk", bufs=2) as tkpool,
            tc.tile_pool(name="psum", bufs=1, space=bass.MemorySpace.PSUM) as pp,
        ):
            Mt = pool.tile([P, CH, K, AUG], f32)     # [Om | Sg | mu] -> [G | Z | c]
            sT = pool.tile([P, CH, K, K], f32)       # Sg (dense-loaded; LU pivots in place)
            omS = pool.tile([P, CH, K, K], f32)      # Om staging (dense DMA)
            muS = pool.tile([P, CH, K], f32)         # mu staging
            OUTt = pool.tile([P, CH, K, AUG], f32)   # outer-product scratch
            OUT2 = pool.tile([P, CH, K - 1, K - 1], f32)
            wt = pool.tile([P, A2, M], f32)
            cstT = pool.tile([M, 2, SW], f32)
            dT = pool.tile([P, CH], f32)
            dnT = pool.tile([P, CH], f32)
            d2T = pool.tile([P, CH], f32)
            RsT = pool.tile([P, CH, AUG], f32)
            CsT = pool.tile([P, CH, K], f32)
            Cs2 = pool.tile([P, CH, K - 1], f32)
            ppO = [pool.tile([P, CH], f32, name=f"ppO{i}", tag=f"ppO{i}") for i in range(2)]
            ppS = [pool.tile([P, CH], f32, name=f"ppS{i}", tag=f"ppS{i}") for i in range(2)]
            gsq = pool.tile([P, CH], f32)
            rS = pool.tile([P, CH], f32)
            r3 = pool.tile([P, CH], f32)
            ghT = pool.tile([P, CH], f32)
            rowW = pool.tile([P, A2, 1], f32)
            laneP = pool.tile([P, CH], f32)
            Pm = pool.tile([M, 2, SW], f32)
            omt = pool.tile([M, 2], f32)
            ps = [pp.tile([M, SW], f32, name=f"ps{b}", tag=f"ps{b}") for b in range(2)]

            # ---- dense loads (contiguous 1KB runs per partition) ----
            for src, br in ((sgp, 0), (sgr, 1)):
                nc.sync.dma_start(sT[:, 2 * br:2 * br + 2, :, :],
                                  src.rearrange("(a p) i j -> p a i j", p=P))
            for src, br in ((omg, 0), (omm, 1)):
                nc.sync.dma_start(omS[:, 2 * br:2 * br + 2, :, :],
                                  src.rearrange("(a p) i j -> p a i j", p=P))
            for src, br in ((mup, 0), (mur, 1)):
                nc.sync.dma_start(muS[:, 2 * br:2 * br + 2, :],
                                  src.rearrange("(a p) i -> p a i", p=P))
            nc.sync.dma_start(wt[:, :, :], win.rearrange("(a p) m -> p a m", p=P))
            nc.sync.dma_start(cstT[:, :, :], cst.rearrange("b m e -> m b e"))

            nc.vector.memset(ppO[0][:, :], 1.0)
            nc.vector.memset(ppS[0][:, :], 1.0)

            # ---- assemble augmented tile on the scalar engine (off DVE) ----
            nc.scalar.copy(Mt[:, :, :, K:2 * K], sT[:, :, :, :])
            nc.scalar.copy(Mt[:, :, :, 0:K], omS[:, :, :, :])
            nc.scalar.copy(Mt[:, :, :, 2 * K], muS[:, :, :])

            # ---- LU pivot products of Sg (logdet); first on DVE to overlap
            # with the DMA/copy assembly of Mt ----
            for i in range(K):
                piv = sT[:, :, i, i]
                nc.vector.tensor_tensor(
                    ppS[(i + 1) % 2][:, :], ppS[i % 2][:, :], piv, Alu.mult
                )
                if i == K - 1:
                    break
                r = K - 1 - i
                nc.vector.reciprocal(d2T[:, :], piv)
                nc.vector.tensor_tensor(
                    Cs2[:, :, 0:r], sT[:, :, i + 1:, i],
                    d2T[:, :].unsqueeze(2).to_broadcast([P, CH, r]), Alu.mult
                )
                nc.vector.tensor_tensor(
                    OUT2[:, :, 0:r, 0:r],
                    Cs2[:, :, 0:r].unsqueeze(3).to_broadcast([P, CH, r, r]),
                    sT[:, :, i, i + 1:].unsqueeze(2).to_broadcast([P, CH, r, r]),
                    Alu.mult,
                )
                nc.vector.tensor_tensor(
                    sT[:, :, i + 1:, i + 1:], sT[:, :, i + 1:, i + 1:],
                    OUT2[:, :, 0:r, 0:r], Alu.subtract
                )

            # ---- Gauss-Jordan sweep on [Om | Sg | mu]: -> [G | G Sg | G mu] ----
            Mflat = Mt[:, :, :, :].rearrange("p c i j -> p (c i j)")
            Oflat = OUTt[:, :, :, :].rearrange("p c i j -> p (c i j)")
            for i in range(K):
                piv = Mt[:, :, i, i]
                nc.vector.tensor_tensor(
                    ppO[(i + 1) % 2][:, :], ppO[i % 2][:, :], piv, Alu.mult
                )
                nc.vector.reciprocal(dT[:, :], piv)
                nc.vector.tensor_scalar_mul(dnT[:, :], dT[:, :], -1.0)
                nc.vector.tensor_tensor(
                    RsT[:, :, :], Mt[:, :, i, :],
                    dT[:, :].unsqueeze(2).to_broadcast([P, CH, AUG]), Alu.mult
                )
                nc.vector.tensor_tensor(
                    CsT[:, :, :], Mt[:, :, :, i],
                    dnT[:, :].unsqueeze(2).to_broadcast([P, CH, K]), Alu.mult
                )
                nc.vector.tensor_tensor(
                    OUTt[:, :, :, :],
                    Mt[:, :, :, i].unsqueeze(3).to_broadcast([P, CH, K, AUG]),
                    RsT[:, :, :].unsqueeze(2).to_broadcast([P, CH, K, AUG]),
                    Alu.mult,
                )
                nc.vector.tensor_tensor(Mflat, Mflat, Oflat, Alu.subtract)
                nc.scalar.copy(Mt[:, :, :, i], CsT[:, :, :])
                nc.scalar.copy(Mt[:, :, i, :], RsT[:, :, :])
                nc.scalar.copy(Mt[:, :, i, i], dT[:, :])

            # ---- g - h = log(det(Om)^2 / det(Sg)) ----
            nc.vector.tensor_tensor(gsq[:, :], ppO[0][:, :], ppO[0][:, :], Alu.mult)
            nc.vector.reciprocal(rS[:, :], ppS[0][:, :])
            nc.vector.tensor_tensor(r3[:, :], gsq[:, :], rS[:, :], Alu.mult)
            nc.scalar.activation(ghT[:, :], r3[:, :], Act.Ln)

            # ---- lane partials: rowsum(W) * (g - h) ----
            nc.vector.tensor_reduce(
                rowW[:, :, :], wt[:, :, :], mybir.AxisListType.X, Alu.add
            )
            nc.vector.tensor_tensor(
                laneP[:, :].rearrange("p (b a) -> p b a", b=2),
                ghT[:, :].rearrange("p (b a) -> p b a", b=2),
                rowW[:, :, 0].unsqueeze(1).to_broadcast([P, 2, A2]),
                Alu.mult,
            )
            nc.sync.dma_start(out_lane[:, :], laneP[:, :])

            # ---- S-terms streamed through PE with PSUM accumulation:
            #   P_m[:, 0:256] = sum_k sum_n W[n,m] * (Zcol_k (x) Gcol_k + c (x) c)
            #   P_m[:, 256:272] = sum_n W[n,m] * c
            for br in range(2):
                for k in range(K + 1):
                    zo = 2 * K if k == K else K + k    # Z col k, or c
                    go = 2 * K if k == K else k        # G col k, or c
                    cc = slice(2 * br, 2 * br + 2)
                    Tk = tkpool.tile([P, A2, K * K], f32, name="Tk", tag="Tk")
                    nc.vector.tensor_tensor(
                        Tk[:, :, :].rearrange("p c (i j) -> p c i j", i=K),
                        Mt[:, cc, :, zo].unsqueeze(3).to_broadcast([P, A2, K, K]),
                        Mt[:, cc, :, go].unsqueeze(2).to_broadcast([P, A2, K, K]),
                        Alu.mult,
                    )
                    for a in range(A2):
                        nc.tensor.matmul(
                            ps[br][:, 0:K * K], wt[:, a, :], Tk[:, a, :],
                            start=(k == 0 and a == 0),
                            stop=(k == K and a == A2 - 1),
                        )
                for a in range(A2):
                    nc.tensor.matmul(
                        ps[br][:, K * K:SW], wt[:, a, :],
                        Mt[:, 2 * br + a, :, 2 * K],
                        start=(a == 0), stop=(a == A2 - 1),
                    )
            for br in range(2):
                nc.vector.tensor_tensor(
                    Pm[:, br, :], ps[br][:, :], cstT[:, br, :], Alu.mult
                )
                nc.vector.tensor_reduce(
                    omt[:, br:br + 1], Pm[:, br, :], mybir.AxisListType.X, Alu.add
                )
            nc.sync.dma_start(out_m.rearrange("b m -> m b"), omt[:, :])

    nc.compile()
    return nc


def _get_program():
    global _PROG
    if _PROG is None:
        _PROG = _build_program()
    return _PROG


def _parent_precompute(mu_p, sg_p, om_p):
    mu = mu_p.astype(np.float64)
    sg = sg_p.astype(np.float64)
    om = om_p.astype(np.float64)
    Si = np.linalg.inv(sg)
    A = np.einsum('mji,mjk,mkl->mil', om, Si, om)
    e = np.einsum('mji,mjk,mk->mi', om, Si, mu)
    d = np.einsum('mj,mjk,mk->m', mu, Si, mu)
    f = np.linalg.slogdet(sg)[1] - 2.0 * np.log(np.abs(np.linalg.det(om)))
    cst = np.concatenate([A.reshape(M, K * K), -2.0 * e], axis=1).astype(np.float32)
    return cst, d, f


def make_in_maps(inputs):
    """Shard inputs over cores + build replicated parent constants.
    Returns (in_maps, host_terms[2])."""
    cb, db, fb = _parent_precompute(
        inputs['mu_q_parent'], inputs['sigma_q_parent'], inputs['omega_parent'])
    cm, dm, fm = _parent_precompute(
        inputs['mu_s_parent'], inputs['sigma_s_parent'], inputs['omega_m_parent'])
    cst = np.stack([cb, cm]).astype(np.float32)                      # (2, M, SW)
    W64 = inputs['W'].astype(np.float64)
    colW = W64.sum(axis=0)
    host_terms = [float((colW * (db + fb - K)).sum()),
                  float((colW * (dm + fm - K)).sum())]

    f32c = lambda x: np.ascontiguousarray(x, dtype=np.float32)
    in_maps = []
    for c in range(NC):
        s = slice(c * NS, (c + 1) * NS)
        in_maps.append({
            'omg': f32c(inputs['omega_child'][s]),
            'omm': f32c(inputs['omega_m_child'][s]),
            'sgp': f32c(inputs['sigma_p'][s]),
            'sgr': f32c(inputs['sigma_r'][s]),
            'mup': f32c(inputs['mu_p'][s]),
            'mur': f32c(inputs['mu_r'][s]),
            'w': f32c(inputs['W'][s]),
            'cst': cst,
        })
    return in_maps, host_terms


def assemble(results, host_terms):
    vals = []
    for br in range(2):
        dev = 0.0
        for r in results:
            dev += float(r['out_m'][br].astype(np.float64).sum())
            dev += float(r['out_lane'][:, 2 * br:2 * br + 2].astype(np.float64).sum())
        vals.append(0.5 * (dev + host_terms[br]))
    belief, model = vals
    total = belief + model
    return (np.float32(total), np.float32(belief), np.float32(model))


def kernel(**inputs):
    from concourse import bass_utils
    nc = _get_program()
    in_maps, host_terms = make_in_maps(inputs)
    res = bass_utils.run_bass_kernel_spmd(nc, in_maps, core_ids=list(range(NC)))
    return assemble(res.results, host_terms)


if __name__ == "__main__":
    _build_program()
    print("program builds OK")


# revision 6
# speedup vs baseline: 1.1986x; 1.0447x over previous
"""CrossScaleVFE kernel for 8x Trainium2 NeuronCores.

Math (per branch, belief/model):
  total = sum_{n,m} W[n,m] * KL( N(mu_c[n], Sg_c[n]) || N(T mu_p[m], T Sg_p[m] T^T) ),
  T = Om_c[n] @ Om_p[m]^-1.

Reduction used here (G = Om_c^-1, c = G mu_c, B = G Sg_c G^T, S = B + c c^T):
  KL[n,m] = 0.5*( <A_m, S_n> - 2 e_m . c_n + d_m - K + f_m + g_n - h_n )
  A_m = Om_p^T Sg_p^-1 Om_p, e_m = Om_p^T Sg_p^-1 mu_p, d_m = mu_p^T Sg_p^-1 mu_p,
  f_m = logdet Sg_p - 2 log|det Om_p|, g_n = 2 log|det Om_c|, h_n = logdet Sg_c.

Sharding: child axis N split over 8 cores (256 children/core); parent-derived
constants (64 parents) replicated. Device computes, per core:
  - batched 16x16 Gauss-Jordan of [Om | Sg | mu] -> G, Z=G Sg, c=G mu + det(Om)
  - batched LU pivots of Sg -> det(Sg)
  - S = Z G^T + c c^T; PE matmul P = W^T [S | c]; out_m = reduce(P * [A | -2e])
  - out_lane = rowsum(W) * (g - h)
Host does the tiny per-parent precompute and final scalar assembly.
"""
import numpy as np

N, M, K, NC = 2048, 64, 16, 8
NS = N // NC          # children per core
P = 128               # partitions
A2 = NS // P          # child chunks per branch (2)
CH = 2 * A2           # total chunks (belief a0,a1, model a0,a1)
AUG = 2 * K + 1       # 33: [Om | Sg | mu]
SW = K * K + K        # 272: [S_flat | c]

_PROG = None


def _build_program():
    import concourse.bass as bass
    import concourse.bacc as bacc
    import concourse.tile as tile
    import concourse.mybir as mybir

    f32 = mybir.dt.float32
    Alu = mybir.AluOpType
    Act = mybir.ActivationFunctionType

    nc = bacc.Bacc("TRN2", target_bir_lowering=False, debug=False, num_devices=NC)

    omg = nc.dram_tensor("omg", (NS, K, K), f32, kind="ExternalInput").ap()
    omm = nc.dram_tensor("omm", (NS, K, K), f32, kind="ExternalInput").ap()
    sgp = nc.dram_tensor("sgp", (NS, K, K), f32, kind="ExternalInput").ap()
    sgr = nc.dram_tensor("sgr", (NS, K, K), f32, kind="ExternalInput").ap()
    mup = nc.dram_tensor("mup", (NS, K), f32, kind="ExternalInput").ap()
    mur = nc.dram_tensor("mur", (NS, K), f32, kind="ExternalInput").ap()
    win = nc.dram_tensor("w", (NS, M), f32, kind="ExternalInput").ap()
    cst = nc.dram_tensor("cst", (2, M, SW), f32, kind="ExternalInput").ap()
    out_m = nc.dram_tensor("out_m", (2, M), f32, kind="ExternalOutput").ap()
    out_lane = nc.dram_tensor("out_lane", (P, CH), f32, kind="ExternalOutput").ap()

    with tile.TileContext(nc) as tc:
        with (
            tc.tile_pool(name="main", bufs=1) as pool,
            tc.tile_pool(name="tk", bufs=2) as tkpool,
            tc.tile_pool(name="psum", bufs=1, space=bass.MemorySpace.PSUM) as pp,
        ):
            Mt = pool.tile([P, CH, K, AUG], f32)     # [Om | Sg | mu] -> [G | Z | c]
            sT = pool.tile([P, CH, K, K], f32)       # Sg (dense-loaded; LU pivots in place)
            omS = pool.tile([P, CH, K, K], f32)      # Om staging (dense DMA)
            muS = pool.tile([P, CH, K], f32)         # mu staging
            OUTt = pool.tile([P, CH, K, AUG], f32)   # outer-product scratch
            OUT2 = pool.tile([P, CH, K - 1, K - 1], f32)
            wt = pool.tile([P, A2, M], f32)
            cstT = pool.tile([M, 2, SW], f32)
            dT = pool.tile([P, CH], f32)
            dnT = pool.tile([P, CH], f32)
            d2T = pool.tile([P, CH], f32)
            RsT = pool.tile([P, CH, AUG], f32)
            CsT = pool.tile([P, CH, K], f32)
            Cs2 = pool.tile([P, CH, K - 1], f32)
            Gcols = pool.tile([P, CH, K + 1, K], f32)  # G cols + c, contiguous
            piv16 = pool.tile([P, CH, K], f32)         # 1/pivot_i of the sweep
            absd = pool.tile([P, CH, K], f32)
            lnd = pool.tile([P, CH, K], f32)
            lnS = pool.tile([P, CH, K], f32)
            red1 = pool.tile([P, CH, 1], f32)
            red2 = pool.tile([P, CH, 1], f32)
            ghT = pool.tile([P, CH], f32)
            rowW = pool.tile([P, A2, 1], f32)
            laneP = pool.tile([P, CH], f32)
            Pm = pool.tile([M, 2, SW], f32)
            omt = pool.tile([M, 2], f32)
            ps = [pp.tile([M, SW], f32, name=f"ps{b}", tag=f"ps{b}") for b in range(2)]

            # ---- dense loads (contiguous 1KB runs per partition) ----
            for src, br in ((sgp, 0), (sgr, 1)):
                nc.sync.dma_start(sT[:, 2 * br:2 * br + 2, :, :],
                                  src.rearrange("(a p) i j -> p a i j", p=P))
            for src, br in ((omg, 0), (omm, 1)):
                nc.sync.dma_start(omS[:, 2 * br:2 * br + 2, :, :],
                                  src.rearrange("(a p) i j -> p a i j", p=P))
            for src, br in ((mup, 0), (mur, 1)):
                nc.sync.dma_start(muS[:, 2 * br:2 * br + 2, :],
                                  src.rearrange("(a p) i -> p a i", p=P))
            nc.sync.dma_start(wt[:, :, :], win.rearrange("(a p) m -> p a m", p=P))
            nc.sync.dma_start(cstT[:, :, :], cst.rearrange("b m e -> m b e"))

            # ---- assemble augmented tile on the scalar engine (off DVE) ----
            nc.scalar.copy(Mt[:, :, :, K:2 * K], sT[:, :, :, :])
            nc.scalar.copy(Mt[:, :, :, 0:K], omS[:, :, :, :])
            nc.scalar.copy(Mt[:, :, :, 2 * K], muS[:, :, :])

            # ---- LU pivot products of Sg (logdet); first on DVE to overlap
            # with the DMA/copy assembly of Mt ----
            for i in range(K - 1):
                piv = sT[:, :, i, i]
                r = K - 1 - i
                nc.vector.reciprocal(d2T[:, :], piv)
                nc.vector.tensor_tensor(
                    Cs2[:, :, 0:r], sT[:, :, i + 1:, i],
                    d2T[:, :].unsqueeze(2).to_broadcast([P, CH, r]), Alu.mult
                )
                nc.vector.tensor_tensor(
                    OUT2[:, :, 0:r, 0:r],
                    Cs2[:, :, 0:r].unsqueeze(3).to_broadcast([P, CH, r, r]),
                    sT[:, :, i, i + 1:].unsqueeze(2).to_broadcast([P, CH, r, r]),
                    Alu.mult,
                )
                nc.vector.tensor_tensor(
                    sT[:, :, i + 1:, i + 1:], sT[:, :, i + 1:, i + 1:],
                    OUT2[:, :, 0:r, 0:r], Alu.subtract
                )

            # ---- Gauss-Jordan sweep on [Om | Sg | mu]: -> [G | G Sg | G mu] ----
            Mflat = Mt[:, :, :, :].rearrange("p c i j -> p (c i j)")
            Oflat = OUTt[:, :, :, :].rearrange("p c i j -> p (c i j)")
            for i in range(K):
                piv = Mt[:, :, i, i]
                nc.vector.reciprocal(dT[:, :], piv)
                nc.scalar.copy(piv16[:, :, i], dT[:, :])
                nc.vector.tensor_tensor(
                    RsT[:, :, :], Mt[:, :, i, :],
                    dT[:, :].unsqueeze(2).to_broadcast([P, CH, AUG]), Alu.mult
                )
                nc.vector.tensor_tensor(
                    CsT[:, :, :], Mt[:, :, :, i],
                    dT[:, :].unsqueeze(2).to_broadcast([P, CH, K]), Alu.mult
                )
                nc.vector.tensor_tensor(
                    OUTt[:, :, :, :],
                    Mt[:, :, :, i].unsqueeze(3).to_broadcast([P, CH, K, AUG]),
                    RsT[:, :, :].unsqueeze(2).to_broadcast([P, CH, K, AUG]),
                    Alu.mult,
                )
                nc.vector.tensor_tensor(Mflat, Mflat, Oflat, Alu.subtract)
                nc.scalar.activation(Mt[:, :, :, i], CsT[:, :, :], Act.Copy, scale=-1.0)
                nc.scalar.copy(Mt[:, :, i, :], RsT[:, :, :])
                nc.scalar.copy(Mt[:, :, i, i], dT[:, :])

            # ---- g - h = -2 sum_i ln|1/p_i^Om| - sum_j ln p_j^Sg ----
            nc.scalar.activation(absd[:, :, :], piv16[:, :, :], Act.Abs)
            nc.scalar.activation(lnd[:, :, :], absd[:, :, :], Act.Ln)
            nc.scalar.activation(
                lnS[:, :, :],
                sT[:, :, :, :].rearrange("p c i j -> p c (i j)")[:, :, ::K + 1],
                Act.Ln)
            nc.vector.tensor_reduce(red1[:, :, :], lnd[:, :, :], mybir.AxisListType.X, Alu.add)
            nc.vector.tensor_reduce(red2[:, :, :], lnS[:, :, :], mybir.AxisListType.X, Alu.add)
            nc.vector.scalar_tensor_tensor(
                ghT[:, :], red1[:, :, 0], -2.0, red2[:, :, 0],
                Alu.mult, Alu.subtract)

            # ---- lane partials: rowsum(W) * (g - h) ----
            nc.vector.tensor_reduce(
                rowW[:, :, :], wt[:, :, :], mybir.AxisListType.X, Alu.add
            )
            nc.vector.tensor_tensor(
                laneP[:, :].rearrange("p (b a) -> p b a", b=2),
                ghT[:, :].rearrange("p (b a) -> p b a", b=2),
                rowW[:, :, 0].unsqueeze(1).to_broadcast([P, 2, A2]),
                Alu.mult,
            )
            nc.sync.dma_start(out_lane[:, :], laneP[:, :])

            # ---- S-terms streamed through PE with PSUM accumulation:
            #   P_m[:, 0:256] = sum_k sum_n W[n,m] * (Zcol_k (x) Gcol_k + c (x) c)
            #   P_m[:, 256:272] = sum_n W[n,m] * c
            nc.scalar.copy(Gcols[:, :, 0:K, :],
                           Mt[:, :, :, 0:K].rearrange("p c i k -> p c k i"))
            nc.scalar.copy(Gcols[:, :, K, :], Mt[:, :, :, 2 * K])
            for br in range(2):
                for k in range(K + 1):
                    zo = 2 * K if k == K else K + k    # Z col k, or c
                    kk = k if k < K else K             # G col k, or c
                    cc = slice(2 * br, 2 * br + 2)
                    Tk = tkpool.tile([P, A2, K * K], f32, name="Tk", tag="Tk")
                    nc.vector.tensor_tensor(
                        Tk[:, :, :].rearrange("p c (i j) -> p c i j", i=K),
                        Mt[:, cc, :, zo].unsqueeze(3).to_broadcast([P, A2, K, K]),
                        Gcols[:, cc, kk, :].unsqueeze(2).to_broadcast([P, A2, K, K]),
                        Alu.mult,
                    )
                    for a in range(A2):
                        nc.tensor.matmul(
                            ps[br][:, 0:K * K], wt[:, a, :], Tk[:, a, :],
                            start=(k == 0 and a == 0),
                            stop=(k == K and a == A2 - 1),
                        )
                for a in range(A2):
                    nc.tensor.matmul(
                        ps[br][:, K * K:SW], wt[:, a, :],
                        Mt[:, 2 * br + a, :, 2 * K],
                        start=(a == 0), stop=(a == A2 - 1),
                    )
            for br in range(2):
                nc.vector.tensor_tensor(
                    Pm[:, br, :], ps[br][:, :], cstT[:, br, :], Alu.mult
                )
                nc.vector.tensor_reduce(
                    omt[:, br:br + 1], Pm[:, br, :], mybir.AxisListType.X, Alu.add
                )
            nc.sync.dma_start(out_m.rearrange("b m -> m b"), omt[:, :])

    nc.compile()
    return nc


def _get_program():
    global _PROG
    if _PROG is None:
        _PROG = _build_program()
    return _PROG


def _parent_precompute(mu_p, sg_p, om_p):
    mu = mu_p.astype(np.float64)
    sg = sg_p.astype(np.float64)
    om = om_p.astype(np.float64)
    Si = np.linalg.inv(sg)
    A = np.einsum('mji,mjk,mkl->mil', om, Si, om)
    e = np.einsum('mji,mjk,mk->mi', om, Si, mu)
    d = np.einsum('mj,mjk,mk->m', mu, Si, mu)
    f = np.linalg.slogdet(sg)[1] - 2.0 * np.log(np.abs(np.linalg.det(om)))
    cst = np.concatenate([A.reshape(M, K * K), -2.0 * e], axis=1).astype(np.float32)
    return cst, d, f


def make_in_maps(inputs):
    """Shard inputs over cores + build replicated parent constants.
    Returns (in_maps, host_terms[2])."""
    cb, db, fb = _parent_precompute(
        inputs['mu_q_parent'], inputs['sigma_q_parent'], inputs['omega_parent'])
    cm, dm, fm = _parent_precompute(
        inputs['mu_s_parent'], inputs['sigma_s_parent'], inputs['omega_m_parent'])
    cst = np.stack([cb, cm]).astype(np.float32)                      # (2, M, SW)
    W64 = inputs['W'].astype(np.float64)
    colW = W64.sum(axis=0)
    host_terms = [float((colW * (db + fb - K)).sum()),
                  float((colW * (dm + fm - K)).sum())]

    f32c = lambda x: np.ascontiguousarray(x, dtype=np.float32)
    in_maps = []
    for c in range(NC):
        s = slice(c * NS, (c + 1) * NS)
        in_maps.append({
            'omg': f32c(inputs['omega_child'][s]),
            'omm': f32c(inputs['omega_m_child'][s]),
            'sgp': f32c(inputs['sigma_p'][s]),
            'sgr': f32c(inputs['sigma_r'][s]),
            'mup': f32c(inputs['mu_p'][s]),
            'mur': f32c(inputs['mu_r'][s]),
            'w': f32c(inputs['W'][s]),
            'cst': cst,
        })
    return in_maps, host_terms


def assemble(results, host_terms):
    vals = []
    for br in range(2):
        dev = 0.0
        for r in results:
            dev += float(r['out_m'][br].astype(np.float64).sum())
            dev += float(r['out_lane'][:, 2 * br:2 * br + 2].astype(np.float64).sum())
        vals.append(0.5 * (dev + host_terms[br]))
    belief, model = vals
    total = belief + model
    return (np.float32(total), np.float32(belief), np.float32(model))


def kernel(**inputs):
    from concourse import bass_utils
    nc = _get_program()
    in_maps, host_terms = make_in_maps(inputs)
    res = bass_utils.run_bass_kernel_spmd(nc, in_maps, core_ids=list(range(NC)))
    return assemble(res.results, host_terms)


if __name__ == "__main__":
    _build_program()
    print("program builds OK")


# revision 7
# speedup vs baseline: 1.2301x; 1.0264x over previous
"""CrossScaleVFE kernel for 8x Trainium2 NeuronCores.

Math (per branch, belief/model):
  total = sum_{n,m} W[n,m] * KL( N(mu_c[n], Sg_c[n]) || N(T mu_p[m], T Sg_p[m] T^T) ),
  T = Om_c[n] @ Om_p[m]^-1.

Reduction used here (G = Om_c^-1, c = G mu_c, B = G Sg_c G^T, S = B + c c^T):
  KL[n,m] = 0.5*( <A_m, S_n> - 2 e_m . c_n + d_m - K + f_m + g_n - h_n )
  A_m = Om_p^T Sg_p^-1 Om_p, e_m = Om_p^T Sg_p^-1 mu_p, d_m = mu_p^T Sg_p^-1 mu_p,
  f_m = logdet Sg_p - 2 log|det Om_p|, g_n = 2 log|det Om_c|, h_n = logdet Sg_c.

Sharding: child axis N split over 8 cores (256 children/core); parent-derived
constants (64 parents) replicated. Device computes, per core:
  - batched 16x16 Gauss-Jordan of [Om | Sg | mu] -> G, Z=G Sg, c=G mu + det(Om)
  - batched LU pivots of Sg -> det(Sg)
  - S = Z G^T + c c^T; PE matmul P = W^T [S | c]; out_m = reduce(P * [A | -2e])
  - out_lane = rowsum(W) * (g - h)
Host does the tiny per-parent precompute and final scalar assembly.
"""
import numpy as np

N, M, K, NC = 2048, 64, 16, 8
NS = N // NC          # children per core
P = 128               # partitions
A2 = NS // P          # child chunks per branch (2)
CH = 2 * A2           # total chunks (belief a0,a1, model a0,a1)
AUG = 2 * K + 1       # 33: [Om | Sg | mu]
SW = K * K + K        # 272: [S_flat | c]

_PROG = None


def _build_program():
    import concourse.bass as bass
    import concourse.bacc as bacc
    import concourse.tile as tile
    import concourse.mybir as mybir

    f32 = mybir.dt.float32
    Alu = mybir.AluOpType
    Act = mybir.ActivationFunctionType

    nc = bacc.Bacc("TRN2", target_bir_lowering=False, debug=False, num_devices=NC)

    omg = nc.dram_tensor("omg", (NS, K, K), f32, kind="ExternalInput").ap()
    omm = nc.dram_tensor("omm", (NS, K, K), f32, kind="ExternalInput").ap()
    sgp = nc.dram_tensor("sgp", (NS, K, K), f32, kind="ExternalInput").ap()
    sgr = nc.dram_tensor("sgr", (NS, K, K), f32, kind="ExternalInput").ap()
    mup = nc.dram_tensor("mup", (NS, K), f32, kind="ExternalInput").ap()
    mur = nc.dram_tensor("mur", (NS, K), f32, kind="ExternalInput").ap()
    win = nc.dram_tensor("w", (NS, M), f32, kind="ExternalInput").ap()
    cst = nc.dram_tensor("cst", (2, M, SW), f32, kind="ExternalInput").ap()
    out_m = nc.dram_tensor("out_m", (2, M), f32, kind="ExternalOutput").ap()
    out_lane = nc.dram_tensor("out_lane", (P, CH), f32, kind="ExternalOutput").ap()

    with tile.TileContext(nc) as tc:
        with (
            tc.tile_pool(name="main", bufs=1) as pool,
            tc.tile_pool(name="tk", bufs=2) as tkpool,
            tc.tile_pool(name="psum", bufs=1, space=bass.MemorySpace.PSUM) as pp,
        ):
            Mt = pool.tile([P, CH, K, AUG], f32)     # [Om | Sg | mu] -> [G | Z | c]
            sT = pool.tile([P, CH, K, K], f32)       # Sg (dense-loaded; LU pivots in place)
            omS = pool.tile([P, CH, K, K], f32)      # Om staging (dense DMA)
            muS = pool.tile([P, CH, K], f32)         # mu staging
            OUTt = pool.tile([P, CH, K, AUG], f32)   # outer-product scratch
            OUT2 = pool.tile([P, CH, K - 1, K - 1], f32)
            wt = pool.tile([P, A2, M], f32)
            cstT = pool.tile([M, 2, SW], f32)
            dT = pool.tile([P, CH], f32)
            d2T = pool.tile([P, CH], f32)
            RsT = pool.tile([P, CH, AUG], f32)
            Cs2 = pool.tile([P, CH, K - 1], f32)
            Gcols = pool.tile([P, CH, K + 1, K], f32)  # G cols + c, contiguous
            piv16 = pool.tile([P, CH, K], f32)         # 1/pivot_i of the sweep
            absd = pool.tile([P, CH, K], f32)
            lnd = pool.tile([P, CH, K], f32)
            lnS = pool.tile([P, CH, K], f32)
            red1 = pool.tile([P, CH, 1], f32)
            red2 = pool.tile([P, CH, 1], f32)
            ghT = pool.tile([P, CH], f32)
            rowW = pool.tile([P, A2, 1], f32)
            laneP = pool.tile([P, CH], f32)
            Pm = pool.tile([M, 2, SW], f32)
            omt = pool.tile([M, 2], f32)
            ps = [pp.tile([M, SW], f32, name=f"ps{b}", tag=f"ps{b}") for b in range(2)]

            # ---- dense loads (contiguous 1KB runs per partition) ----
            for src, br in ((sgp, 0), (sgr, 1)):
                nc.sync.dma_start(sT[:, 2 * br:2 * br + 2, :, :],
                                  src.rearrange("(a p) i j -> p a i j", p=P))
            for src, br in ((omg, 0), (omm, 1)):
                nc.sync.dma_start(omS[:, 2 * br:2 * br + 2, :, :],
                                  src.rearrange("(a p) i j -> p a i j", p=P))
            for src, br in ((mup, 0), (mur, 1)):
                nc.sync.dma_start(muS[:, 2 * br:2 * br + 2, :],
                                  src.rearrange("(a p) i -> p a i", p=P))
            nc.sync.dma_start(wt[:, :, :], win.rearrange("(a p) m -> p a m", p=P))
            nc.sync.dma_start(cstT[:, :, :], cst.rearrange("b m e -> m b e"))

            # ---- assemble augmented tile on the scalar engine (off DVE) ----
            nc.scalar.copy(Mt[:, :, :, K:2 * K], sT[:, :, :, :])
            nc.scalar.copy(Mt[:, :, :, 0:K], omS[:, :, :, :])
            nc.scalar.copy(Mt[:, :, :, 2 * K], muS[:, :, :])

            # ---- LU pivot products of Sg (logdet); first on DVE to overlap
            # with the DMA/copy assembly of Mt ----
            for i in range(K - 1):
                piv = sT[:, :, i, i]
                r = K - 1 - i
                nc.vector.reciprocal(d2T[:, :], piv)
                nc.vector.tensor_tensor(
                    Cs2[:, :, 0:r], sT[:, :, i + 1:, i],
                    d2T[:, :].unsqueeze(2).to_broadcast([P, CH, r]), Alu.mult
                )
                nc.vector.tensor_tensor(
                    OUT2[:, :, 0:r, 0:r],
                    Cs2[:, :, 0:r].unsqueeze(3).to_broadcast([P, CH, r, r]),
                    sT[:, :, i, i + 1:].unsqueeze(2).to_broadcast([P, CH, r, r]),
                    Alu.mult,
                )
                nc.vector.tensor_tensor(
                    sT[:, :, i + 1:, i + 1:], sT[:, :, i + 1:, i + 1:],
                    OUT2[:, :, 0:r, 0:r], Alu.subtract
                )

            # ---- Gauss-Jordan sweep on [Om | Sg | mu]: -> [G | G Sg | G mu] ----
            Mflat = Mt[:, :, :, :].rearrange("p c i j -> p (c i j)")
            Oflat = OUTt[:, :, :, :].rearrange("p c i j -> p (c i j)")
            for i in range(K):
                piv = Mt[:, :, i, i]
                nc.vector.reciprocal(dT[:, :], piv)
                nc.scalar.copy(piv16[:, :, i], dT[:, :])
                nc.vector.tensor_tensor(
                    RsT[:, :, :], Mt[:, :, i, :],
                    dT[:, :].unsqueeze(2).to_broadcast([P, CH, AUG]), Alu.mult
                )
                # patch Rs[i] += d: the big SUB then leaves -d*col in col i
                nc.vector.tensor_tensor(
                    RsT[:, :, i:i + 1], RsT[:, :, i:i + 1],
                    dT[:, :].unsqueeze(2), Alu.add
                )
                nc.vector.tensor_tensor(
                    OUTt[:, :, :, :],
                    Mt[:, :, :, i].unsqueeze(3).to_broadcast([P, CH, K, AUG]),
                    RsT[:, :, :].unsqueeze(2).to_broadcast([P, CH, K, AUG]),
                    Alu.mult,
                )
                nc.vector.tensor_tensor(Mflat, Mflat, Oflat, Alu.subtract)
                nc.scalar.copy(Mt[:, :, i, :], RsT[:, :, :])
                nc.scalar.copy(Mt[:, :, i, i], dT[:, :])

            # ---- g - h = -2 sum_i ln|1/p_i^Om| - sum_j ln p_j^Sg ----
            nc.scalar.activation(absd[:, :, :], piv16[:, :, :], Act.Abs)
            nc.scalar.activation(lnd[:, :, :], absd[:, :, :], Act.Ln)
            nc.scalar.activation(
                lnS[:, :, :],
                sT[:, :, :, :].rearrange("p c i j -> p c (i j)")[:, :, ::K + 1],
                Act.Ln)
            nc.vector.tensor_reduce(red1[:, :, :], lnd[:, :, :], mybir.AxisListType.X, Alu.add)
            nc.vector.tensor_reduce(red2[:, :, :], lnS[:, :, :], mybir.AxisListType.X, Alu.add)
            nc.vector.scalar_tensor_tensor(
                ghT[:, :], red1[:, :, 0], -2.0, red2[:, :, 0],
                Alu.mult, Alu.subtract)

            # ---- lane partials: rowsum(W) * (g - h) ----
            nc.vector.tensor_reduce(
                rowW[:, :, :], wt[:, :, :], mybir.AxisListType.X, Alu.add
            )
            nc.vector.tensor_tensor(
                laneP[:, :].rearrange("p (b a) -> p b a", b=2),
                ghT[:, :].rearrange("p (b a) -> p b a", b=2),
                rowW[:, :, 0].unsqueeze(1).to_broadcast([P, 2, A2]),
                Alu.mult,
            )
            nc.sync.dma_start(out_lane[:, :], laneP[:, :])

            # ---- S-terms streamed through PE with PSUM accumulation:
            #   P_m[:, 0:256] = sum_k sum_n W[n,m] * (Zcol_k (x) Gcol_k + c (x) c)
            #   P_m[:, 256:272] = sum_n W[n,m] * c
            nc.scalar.copy(Gcols[:, :, 0:K, :],
                           Mt[:, :, :, 0:K].rearrange("p c i k -> p c k i"))
            nc.scalar.copy(Gcols[:, :, K, :], Mt[:, :, :, 2 * K])
            for br in range(2):
                for k in range(K + 1):
                    zo = 2 * K if k == K else K + k    # Z col k, or c
                    kk = k if k < K else K             # G col k, or c
                    cc = slice(2 * br, 2 * br + 2)
                    Tk = tkpool.tile([P, A2, K * K], f32, name="Tk", tag="Tk")
                    nc.vector.tensor_tensor(
                        Tk[:, :, :].rearrange("p c (i j) -> p c i j", i=K),
                        Mt[:, cc, :, zo].unsqueeze(3).to_broadcast([P, A2, K, K]),
                        Gcols[:, cc, kk, :].unsqueeze(2).to_broadcast([P, A2, K, K]),
                        Alu.mult,
                    )
                    for a in range(A2):
                        nc.tensor.matmul(
                            ps[br][:, 0:K * K], wt[:, a, :], Tk[:, a, :],
                            start=(k == 0 and a == 0),
                            stop=(k == K and a == A2 - 1),
                        )
                for a in range(A2):
                    nc.tensor.matmul(
                        ps[br][:, K * K:SW], wt[:, a, :],
                        Mt[:, 2 * br + a, :, 2 * K],
                        start=(a == 0), stop=(a == A2 - 1),
                    )
            for br in range(2):
                nc.vector.tensor_tensor(
                    Pm[:, br, :], ps[br][:, :], cstT[:, br, :], Alu.mult
                )
                nc.vector.tensor_reduce(
                    omt[:, br:br + 1], Pm[:, br, :], mybir.AxisListType.X, Alu.add
                )
            nc.sync.dma_start(out_m.rearrange("b m -> m b"), omt[:, :])

    nc.compile()
    return nc


def _get_program():
    global _PROG
    if _PROG is None:
        _PROG = _build_program()
    return _PROG


def _parent_precompute(mu_p, sg_p, om_p):
    mu = mu_p.astype(np.float64)
    sg = sg_p.astype(np.float64)
    om = om_p.astype(np.float64)
    Si = np.linalg.inv(sg)
    A = np.einsum('mji,mjk,mkl->mil', om, Si, om)
    e = np.einsum('mji,mjk,mk->mi', om, Si, mu)
    d = np.einsum('mj,mjk,mk->m', mu, Si, mu)
    f = np.linalg.slogdet(sg)[1] - 2.0 * np.log(np.abs(np.linalg.det(om)))
    cst = np.concatenate([A.reshape(M, K * K), -2.0 * e], axis=1).astype(np.float32)
    return cst, d, f


def make_in_maps(inputs):
    """Shard inputs over cores + build replicated parent constants.
    Returns (in_maps, host_terms[2])."""
    cb, db, fb = _parent_precompute(
        inputs['mu_q_parent'], inputs['sigma_q_parent'], inputs['omega_parent'])
    cm, dm, fm = _parent_precompute(
        inputs['mu_s_parent'], inputs['sigma_s_parent'], inputs['omega_m_parent'])
    cst = np.stack([cb, cm]).astype(np.float32)                      # (2, M, SW)
    W64 = inputs['W'].astype(np.float64)
    colW = W64.sum(axis=0)
    host_terms = [float((colW * (db + fb - K)).sum()),
                  float((colW * (dm + fm - K)).sum())]

    f32c = lambda x: np.ascontiguousarray(x, dtype=np.float32)
    in_maps = []
    for c in range(NC):
        s = slice(c * NS, (c + 1) * NS)
        in_maps.append({
            'omg': f32c(inputs['omega_child'][s]),
            'omm': f32c(inputs['omega_m_child'][s]),
            'sgp': f32c(inputs['sigma_p'][s]),
            'sgr': f32c(inputs['sigma_r'][s]),
            'mup': f32c(inputs['mu_p'][s]),
            'mur': f32c(inputs['mu_r'][s]),
            'w': f32c(inputs['W'][s]),
            'cst': cst,
        })
    return in_maps, host_terms


def assemble(results, host_terms):
    vals = []
    for br in range(2):
        dev = 0.0
        for r in results:
            dev += float(r['out_m'][br].astype(np.float64).sum())
            dev += float(r['out_lane'][:, 2 * br:2 * br + 2].astype(np.float64).sum())
        vals.append(0.5 * (dev + host_terms[br]))
    belief, model = vals
    total = belief + model
    return (np.float32(total), np.float32(belief), np.float32(model))


def kernel(**inputs):
    from concourse import bass_utils
    nc = _get_program()
    in_maps, host_terms = make_in_maps(inputs)
    res = bass_utils.run_bass_kernel_spmd(nc, in_maps, core_ids=list(range(NC)))
    return assemble(res.results, host_terms)


if __name__ == "__main__":
    _build_program()
    print("program builds OK")
